# revision 1
# baseline (speedup 1.0000x reference)
"""Gaussian-KDE logsumexp kernel for Trainium2 (8 NeuronCores, SPMD).

Math: out[t] = logsumexp_n( -0.5 * scale[n] * dist2[t, n] ) - Z
with dist2 via the GEMM expansion. Everything folds into ONE matmul by
augmenting the contraction dim:
    xhat[:, t] = [test_t (64), test_sq_t, 1]                       (K = 66)
    yhat[:, n] = [scale_n*train_n (64), -.5*scale_n, -.5*scale_n*train_sq_n - Z]
so  xhat[:, t] . yhat[:, n] = -0.5*scale_n*dist2[t, n] - Z  = exp-argument.
Since weights ~ U[0,1], max_n over the exp-argument is within 1e-5 of -Z for
every t, so exp() never overflows and no per-row max pass is needed; the sum
lands ~e^-67 which is comfortably inside fp32 normal range.

Sharding: test points split 8 ways (256/core); train side replicated.
Per core: 2 t-tiles x [66,128] lhsT vs streamed yhat [66,4096];
ScalarE does exp with free-dim accumulation straight out of PSUM, then ln.
"""

import math
from contextlib import ExitStack

import numpy as np

import concourse.bacc as bacc
import concourse.bass as bass
import concourse.mybir as mybir
import concourse.tile as tile
from concourse.bass_utils import run_bass_kernel_spmd

N_CORES = 8
NT, NTR, D = 2048, 4096, 64
TPC = NT // N_CORES          # 256 test points per core
KA = D + 2                   # 66: augmented contraction dim
P = 128                      # partition tile of test points
T_TILES = TPC // P           # 2
MM_N = 512                   # matmul free-dim (one PSUM bank, fp32)
NB = NTR // MM_N             # 8 rhs blocks resident in SBUF
CHUNK = 2048                 # one ACT exp+accum instruction (4 PSUM banks)
N_CHUNKS = NTR // CHUNK      # 2
F32 = mybir.dt.float32
Z_CONST = float(0.5 * D * math.log(2.0 * math.pi) + math.log(NTR))  # h = 1

# float32r streams 1 col/cycle on the PE (vs 4 for float32); precision is
# validated against the fp32 path in test.py.
MM_DTYPE = mybir.dt.float32r


def build_program(mm_dtype=MM_DTYPE):
    nc = bacc.Bacc("TRN2")
    xh = nc.declare_dram_parameter("xhat", [KA, TPC], mm_dtype, isOutput=False)
    yh = nc.declare_dram_parameter("yhat", [KA, NTR], mm_dtype, isOutput=False)
    out_d = nc.declare_dram_parameter("out", [TPC], F32, isOutput=True)
    out_ap = out_d[:].rearrange("(a b) -> a b", b=1)

    with ExitStack() as ctx:
        tc = ctx.enter_context(tile.TileContext(nc))
        xpool = ctx.enter_context(tc.tile_pool(name="xpool", bufs=1))
        ypool = ctx.enter_context(tc.tile_pool(name="ypool", bufs=1))
        epool = ctx.enter_context(tc.tile_pool(name="epool", bufs=2))
        spool = ctx.enter_context(tc.tile_pool(name="spool", bufs=2))
        psum = ctx.enter_context(tc.tile_pool(name="psum", bufs=2, space="PSUM"))

        negz = xpool.tile([P, 1], F32, tag="negz")
        nc.vector.memset(negz, -Z_CONST)

        xs = xpool.tile([KA, TPC], mm_dtype, tag="xs")
        nc.sync.dma_start(out=xs, in_=xh[:])
        ys = []
        for h in range(N_CHUNKS):
            yt = ypool.tile([KA, CHUNK], mm_dtype, tag=f"y{h}")
            nc.sync.dma_start(out=yt, in_=yh[:, h * CHUNK:(h + 1) * CHUNK])
            ys.append(yt)

        tots = spool.tile([P, T_TILES], F32, tag="tots")
        for it in range(T_TILES):
            sums = spool.tile([P, N_CHUNKS], F32, tag="sums")
            for c in range(N_CHUNKS):
                pt = psum.tile([P, CHUNK], F32, tag="pt")
                for j in range(CHUNK // MM_N):
                    nc.tensor.matmul(
                        pt[:, j * MM_N:(j + 1) * MM_N],
                        xs[:, it * P:(it + 1) * P],
                        ys[c][:, j * MM_N:(j + 1) * MM_N],
                        start=True,
                        stop=True,
                    )
                et = epool.tile([P, CHUNK], F32, tag="et")
                nc.scalar.activation(
                    out=et,
                    in_=pt,
                    func=mybir.ActivationFunctionType.Exp,
                    accum_out=sums[:, c:c + 1],
                )
            nc.vector.reduce_sum(
                out=tots[:, it:it + 1], in_=sums, axis=mybir.AxisListType.X
            )
        # single Ln-table load for everything, then affine -Z (Identity is in
        # the same table set), then per-t-tile stores
        lnt = spool.tile([P, T_TILES], F32, tag="lnt")
        nc.scalar.activation(
            out=lnt, in_=tots, func=mybir.ActivationFunctionType.Ln
        )
        res = spool.tile([P, T_TILES], F32, tag="res")
        nc.scalar.activation(
            out=res,
            in_=lnt,
            func=mybir.ActivationFunctionType.Identity,
            bias=negz,
        )
        for it in range(T_TILES):
            nc.sync.dma_start(
                out=out_ap[it * P:(it + 1) * P, :], in_=res[:, it:it + 1]
            )
    nc.compile()
    return nc


def build_program_raw(mm_dtype=MM_DTYPE):
    """Hand-scheduled raw-Bass version: minimal semaphores, input DMAs and
    the ACT exp-table load hoisted ahead of the boot barrier, no Tile
    drain/barrier tail (the Sync engine finishes last by construction and
    clears the semaphores itself)."""
    nc = bacc.Bacc("TRN2")
    xh = nc.declare_dram_parameter("xhat", [KA, TPC], mm_dtype, isOutput=False)
    yh = nc.declare_dram_parameter("yhat", [KA, NTR], mm_dtype, isOutput=False)
    # [p, it] layout — contiguous DMA from res; host transposes when unsharding
    out_d = nc.declare_dram_parameter("out", [P, T_TILES], F32, isOutput=True)

    YB = 1024                    # columns per y DMA block
    NYB = NTR // YB              # 4
    G = T_TILES * N_CHUNKS       # 4 global chunks

    with ExitStack() as ctx:
        sb = lambda nm, shape, dt: ctx.enter_context(nc.sbuf_tensor(nm, shape, dt))
        xs = sb("xs", [KA, TPC], mm_dtype)
        ys = [sb(f"ys{b}", [KA, YB], mm_dtype) for b in range(NYB)]
        et = [sb(f"et{k}", [P, CHUNK], F32) for k in range(2)]
        dummy_in = sb("dummy_in", [P, 1], F32)
        dummy_out = sb("dummy_out", [P, 1], F32)
        zero = sb("zero", [P, 1], F32)
        negz = sb("negz", [P, 1], F32)
        sums = sb("sums", [P, G], F32)
        tots = sb("tots", [P, T_TILES], F32)
        lnt = sb("lnt", [P, T_TILES], F32)
        res = sb("res", [P, T_TILES], F32)
        pt = [
            ctx.enter_context(nc.psum_tensor(f"pt{k}", [P, CHUNK], F32))
            for k in range(2)
        ]

        sx = ctx.enter_context(nc.semaphore("sx"))
        sy = [ctx.enter_context(nc.semaphore(f"sy{b}")) for b in range(NYB)]
        spe = ctx.enter_context(nc.semaphore("spe"))
        sact = ctx.enter_context(nc.semaphore("sact"))
        svz = ctx.enter_context(nc.semaphore("svz"))
        sv2 = ctx.enter_context(nc.semaphore("sv2"))
        so = ctx.enter_context(nc.semaphore("so"))
        my_sems = [sx, *sy, spe, sact, svz, sv2, so]

        # Sync: input DMAs, issued immediately at boot. Stagger the y blocks
        # in two waves so the first chunk's data gets the full (66-partition
        # port-limited) bandwidth instead of fair-sharing with the later
        # blocks — the first matmuls start ~3us earlier.
        nc.sync.dma_start(out=xs[:], in_=xh[:]).then_inc(sx, 16)
        for b in range(2):
            nc.sync.dma_start(
                out=ys[b][:], in_=yh[:, b * YB:(b + 1) * YB]
            ).then_inc(sy[b], 16)
        nc.sync.wait_ge(sy[0], 16)
        nc.sync.wait_ge(sy[1], 16)
        for b in range(2, NYB):
            nc.sync.dma_start(
                out=ys[b][:], in_=yh[:, b * YB:(b + 1) * YB]
            ).then_inc(sy[b], 16)

        # ACT: trigger the (patched, exp+ln+identity) table load at boot;
        # bias AP is uninitialized garbage, output unused
        nc.scalar.activation(
            dummy_out[:],
            dummy_in[:],
            mybir.ActivationFunctionType.Exp,
            bias=zero[:],
        )

        # DVE: constants
        nc.vector.memset(zero[:], 0.0).then_inc(svz, 1)
        nc.vector.memset(negz[:], -Z_CONST).then_inc(svz, 1)

        # PE stream
        for g in range(G):
            it, c = divmod(g, N_CHUNKS)
            for j in range(CHUNK // MM_N):
                if j == 0 and g >= 2:
                    nc.tensor.wait_ge(sact, g - 1)  # PSUM buf recycled
                if j == 0 and g == 0:
                    nc.tensor.wait_ge(sx, 16)
                if j % 2 == 0:
                    nc.tensor.wait_ge(sy[2 * c + j // 2], 16)
                mm = nc.tensor.matmul(
                    pt[g % 2][:, j * MM_N:(j + 1) * MM_N],
                    xs[:, it * P:(it + 1) * P],
                    ys[2 * c + j // 2][:, (j % 2) * MM_N:(j % 2 + 1) * MM_N],
                    start=True,
                    stop=True,
                )
            mm.then_inc(spe, 1)

        # ACT stream: exp+accumulate per chunk, then ln, -Z, and the result
        # DMAs (ACT is an HWDGE engine, and Sync instructions are slow)
        nc.scalar.wait_ge(svz, 2)
        for g in range(G):
            nc.scalar.wait_ge(spe, g + 1)
            nc.scalar.activation(
                out=et[g % 2][:],
                in_=pt[g % 2][:],
                func=mybir.ActivationFunctionType.Exp,
                bias=zero[:],
                accum_out=sums[:, g:g + 1],
            ).then_inc(sact, 1)

        # DVE: per-t-tile totals
        for it in range(T_TILES):
            nc.vector.wait_ge(sact, N_CHUNKS * (it + 1))
            r = nc.vector.reduce_sum(
                out=tots[:, it:it + 1],
                in_=sums[:, it * N_CHUNKS:(it + 1) * N_CHUNKS],
                axis=mybir.AxisListType.X,
            )
        r.then_inc(sv2, 1)

        nc.scalar.wait_ge(sv2, 1)
        nc.scalar.activation(
            out=lnt[:],
            in_=tots[:],
            func=mybir.ActivationFunctionType.Ln,
            bias=zero[:],
        )
        nc.scalar.activation(
            out=res[:],
            in_=lnt[:],
            func=mybir.ActivationFunctionType.Identity,
            bias=negz[:],
        )
        # single result DMA; out[p, it] = res[p, it]
        nc.scalar.dma_start(out=out_d[:], in_=res[:]).then_inc(so, 16)

        # DVE: wait for the result DMA, then clear all our semaphores in
        # one ranged instruction (all other engines are past their final
        # waits once the out-DMA has completed)
        nc.vector.wait_ge(so, 16)
        sem_nums = sorted(s.num for s in my_sems)
        assert sem_nums == list(range(sem_nums[0], sem_nums[0] + len(sem_nums)))
        nc.vector.sem_clear(range(sem_nums[0], sem_nums[-1] + 1))

    nc.compile()
    # Post-compile surgery: collapse the two ACT table loads into a single
    # load of set 6 (natural_log_exp_and_others: exp + ln + identity), and
    # drop the constructor's const-AP memsets + all-engine boot barrier
    # (nothing reads the const APs; every engine can start immediately).
    _strip_preamble_and_merge_act_tables(nc)
    return nc


def build_program_packed(mm_dtype=MM_DTYPE):
    """Like build_program_raw, but the train-side matrix is transferred at
    full DMA port width: chunk0 (n 0..2047) lands natively as [66, 2048] on
    partitions 0-65, chunk1 (n 2048..4095) lands as a [64, 2048] feature
    block on partitions 64-127 (disjoint ports -> both transfer in
    parallel) plus a tiny [2, 2048] augmented-row block at partitions
    32-33. Chunk1 matmuls are split into a K=64 feat matmul (base 64) and
    a K=2 aug matmul (base 32) accumulating into the same PSUM block."""
    nc = bacc.Bacc("TRN2")
    ya_d = nc.declare_dram_parameter("ya", [KA, CHUNK], mm_dtype, isOutput=False)
    yb_d = nc.declare_dram_parameter("yb", [D, CHUNK], mm_dtype, isOutput=False)
    yba_d = nc.declare_dram_parameter("yba", [2, CHUNK], mm_dtype, isOutput=False)
    xa_d = nc.declare_dram_parameter("xa", [KA, TPC], mm_dtype, isOutput=False)
    xb_d = nc.declare_dram_parameter("xb", [D, TPC], mm_dtype, isOutput=False)
    xba_d = nc.declare_dram_parameter("xba", [2, TPC], mm_dtype, isOutput=False)
    out_d = nc.declare_dram_parameter("out", [P, T_TILES], F32, isOutput=True)

    G = T_TILES * N_CHUNKS       # 4 global chunks

    with ExitStack() as ctx:
        sb = lambda nm, shape, dt: ctx.enter_context(nc.sbuf_tensor(nm, shape, dt))
        ya = sb("ya_s", [KA, CHUNK], mm_dtype)
        yb = sb("yb_s", [P, CHUNK], mm_dtype)       # rows 64..127 used
        yba = sb("yba_s", [34, CHUNK], mm_dtype)    # rows 32..33 used
        xa = sb("xa_s", [KA, TPC], mm_dtype)
        xb = sb("xb_s", [P, TPC], mm_dtype)         # rows 64..127 used
        xba = sb("xba_s", [34, TPC], mm_dtype)      # rows 32..33 used
        et = [sb(f"et{k}", [P, CHUNK], F32) for k in range(2)]
        dummy_in = sb("dummy_in", [P, 1], F32)
        dummy_out = sb("dummy_out", [P, 1], F32)
        zero = sb("zero", [P, 1], F32)
        negz = sb("negz", [P, 1], F32)
        sums = sb("sums", [P, G], F32)
        tots = sb("tots", [P, T_TILES], F32)
        lnt = sb("lnt", [P, T_TILES], F32)
        res = sb("res", [P, T_TILES], F32)
        pt = [
            ctx.enter_context(nc.psum_tensor(f"pt{k}", [P, CHUNK], F32))
            for k in range(2)
        ]

        sya = ctx.enter_context(nc.semaphore("sya"))
        syb = ctx.enter_context(nc.semaphore("syb"))
        syba = ctx.enter_context(nc.semaphore("syba"))
        sxa = ctx.enter_context(nc.semaphore("sxa"))
        sxb = ctx.enter_context(nc.semaphore("sxb"))
        sxba = ctx.enter_context(nc.semaphore("sxba"))
        spe = ctx.enter_context(nc.semaphore("spe"))
        sact = ctx.enter_context(nc.semaphore("sact"))
        svz = ctx.enter_context(nc.semaphore("svz"))
        sv2 = ctx.enter_context(nc.semaphore("sv2"))
        so = ctx.enter_context(nc.semaphore("so"))
        my_sems = [sya, syb, syba, sxa, sxb, sxba, spe, sact, svz, sv2, so]

        # ACT: the critical chunk0 transfer goes first on the ACT queue so it
        # starts at engine boot, before the table load
        nc.scalar.dma_start(out=ya[:], in_=ya_d[:]).then_inc(sya, 16)
        # dummy exp triggers the (patched, exp+ln+identity) table load
        nc.scalar.activation(
            dummy_out[:],
            dummy_in[:],
            mybir.ActivationFunctionType.Exp,
            bias=zero[:],
        )

        # Sync: everything else
        nc.sync.dma_start(out=xa[:], in_=xa_d[:]).then_inc(sxa, 16)
        nc.sync.dma_start(out=yb[D:P, :], in_=yb_d[:]).then_inc(syb, 16)
        nc.sync.dma_start(out=xb[D:P, :], in_=xb_d[:]).then_inc(sxb, 16)
        nc.sync.dma_start(out=yba[32:34, :], in_=yba_d[:]).then_inc(syba, 16)
        nc.sync.dma_start(out=xba[32:34, :], in_=xba_d[:]).then_inc(sxba, 16)

        # DVE: constants
        nc.vector.memset(zero[:], 0.0).then_inc(svz, 1)
        nc.vector.memset(negz[:], -Z_CONST).then_inc(svz, 1)

        # PE stream
        for g in range(G):
            it, c = divmod(g, N_CHUNKS)
            for j in range(CHUNK // MM_N):
                if j == 0 and g >= 2:
                    nc.tensor.wait_ge(sact, g - 1)  # PSUM buf recycled
                blk = pt[g % 2][:, j * MM_N:(j + 1) * MM_N]
                if c == 0:
                    if g == 0 and j == 0:
                        nc.tensor.wait_ge(sya, 16)
                        nc.tensor.wait_ge(sxa, 16)
                    mm = nc.tensor.matmul(
                        blk,
                        xa[:, it * P:(it + 1) * P],
                        ya[:, j * MM_N:(j + 1) * MM_N],
                        start=True,
                        stop=True,
                    )
                else:
                    if g == 1 and j == 0:
                        for s in (syb, sxb, syba, sxba):
                            nc.tensor.wait_ge(s, 16)
                    nc.tensor.matmul(
                        blk,
                        xb[D:P, it * P:(it + 1) * P],
                        yb[D:P, j * MM_N:(j + 1) * MM_N],
                        start=True,
                        stop=False,
                    )
                    mm = nc.tensor.matmul(
                        blk,
                        xba[32:34, it * P:(it + 1) * P],
                        yba[32:34, j * MM_N:(j + 1) * MM_N],
                        start=False,
                        stop=True,
                    )
            mm.then_inc(spe, 1)

        # ACT stream: exp+accumulate per chunk, ln, -Z, result DMA
        nc.scalar.wait_ge(svz, 2)
        for g in range(G):
            nc.scalar.wait_ge(spe, g + 1)
            nc.scalar.activation(
                out=et[g % 2][:],
                in_=pt[g % 2][:],
                func=mybir.ActivationFunctionType.Exp,
                bias=zero[:],
                accum_out=sums[:, g:g + 1],
            ).then_inc(sact, 1)

        for it in range(T_TILES):
            nc.vector.wait_ge(sact, N_CHUNKS * (it + 1))
            r = nc.vector.reduce_sum(
                out=tots[:, it:it + 1],
                in_=sums[:, it * N_CHUNKS:(it + 1) * N_CHUNKS],
                axis=mybir.AxisListType.X,
            )
        r.then_inc(sv2, 1)

        nc.scalar.wait_ge(sv2, 1)
        nc.scalar.activation(
            out=lnt[:],
            in_=tots[:],
            func=mybir.ActivationFunctionType.Ln,
            bias=zero[:],
        )
        nc.scalar.activation(
            out=res[:],
            in_=lnt[:],
            func=mybir.ActivationFunctionType.Identity,
            bias=negz[:],
        )
        nc.scalar.dma_start(out=out_d[:], in_=res[:]).then_inc(so, 16)

        nc.vector.wait_ge(so, 16)
        sem_nums = sorted(s.num for s in my_sems)
        assert sem_nums == list(range(sem_nums[0], sem_nums[0] + len(sem_nums)))
        nc.vector.sem_clear(range(sem_nums[0], sem_nums[-1] + 1))

    nc.compile()
    _strip_preamble_and_merge_act_tables(nc)
    return nc


def _strip_preamble_and_merge_act_tables(nc):
    blk = nc.main_func.blocks[0]
    insts = list(blk.instructions)
    drop = set()
    for k, inst in enumerate(insts):
        tn = type(inst).__name__
        if tn == "InstEventSemaphore" and inst.name.startswith("barrier_"):
            drop.add(inst.name)  # boot-barrier event semaphores
            # ... and the per-engine drain feeding this barrier entry
            if k > 0 and type(insts[k - 1]).__name__ == "InstDrain":
                drop.add(insts[k - 1].name)
        elif tn == "InstMemset" and inst.outs and "const-" in str(inst.outs[0]):
            drop.add(inst.name)  # const-AP memsets (nothing reads the const APs)
    new_insts = []
    first_load_seen = False
    for inst in insts:
        if inst.name in drop:
            continue
        if type(inst).__name__ == "InstLoadActFuncSet":
            if first_load_seen:
                assert not inst.has_wait() and not inst.has_update(), inst.name
                continue
            inst.act_func_set_id = 6
            first_load_seen = True
        new_insts.append(inst)
    blk.instructions[:] = new_insts


_PROG = {}


def _get_prog(mm_dtype=MM_DTYPE, impl="packed"):
    key = (mm_dtype, impl)
    if key not in _PROG:
        builder = {
            "raw": build_program_raw,
            "packed": build_program_packed,
            "tile": build_program,
        }[impl]
        _PROG[key] = builder(mm_dtype)
    return _PROG[key]


def _prepare(test_Xs, train_Xs, weights):
    test_Xs = np.asarray(test_Xs, dtype=np.float32)
    train_Xs = np.asarray(train_Xs, dtype=np.float32)
    weights = np.asarray(weights, dtype=np.float32)

    test_sq = (test_Xs.astype(np.float64) ** 2).sum(1)
    train_sq = (train_Xs.astype(np.float64) ** 2).sum(1)
    scale = weights.astype(np.float64) ** 2

    xhat = np.empty((KA, NT), np.float32)
    xhat[:D] = test_Xs.T
    xhat[D] = test_sq
    xhat[D + 1] = 1.0

    yhat = np.empty((KA, NTR), np.float32)
    yhat[:D] = (train_Xs.astype(np.float64) * scale[:, None]).T
    yhat[D] = -0.5 * scale
    yhat[D + 1] = -0.5 * scale * train_sq
    return xhat, yhat


def kernel(test_Xs, train_Xs, weights, mm_dtype=MM_DTYPE, trace=False,
           impl="packed"):
    xhat, yhat = _prepare(test_Xs, train_Xs, weights)
    nc = _get_prog(mm_dtype, impl)
    if impl == "packed":
        ya = np.ascontiguousarray(yhat[:, :CHUNK])
        yb = np.ascontiguousarray(yhat[:D, CHUNK:])
        yba = np.ascontiguousarray(yhat[D:, CHUNK:])
        in_maps = []
        for c in range(N_CORES):
            xa = np.ascontiguousarray(xhat[:, c * TPC:(c + 1) * TPC])
            in_maps.append({
                "ya": ya, "yb": yb, "yba": yba,
                "xa": xa,
                "xb": np.ascontiguousarray(xa[:D]),
                "xba": np.ascontiguousarray(xa[D:]),
            })
    else:
        in_maps = [
            {"xhat": np.ascontiguousarray(xhat[:, c * TPC:(c + 1) * TPC]),
             "yhat": yhat}
            for c in range(N_CORES)
        ]
    res = run_bass_kernel_spmd(nc, in_maps, list(range(N_CORES)), trace=trace)
    parts = []
    for c in range(N_CORES):
        o = res.results[c]["out"]
        parts.append(o.T.ravel() if o.ndim == 2 else o)
    out = np.concatenate(parts)
    if trace:
        kernel.last_results = res
    return out



# revision 4
# speedup vs baseline: 1.8533x; 1.8533x over previous
"""Gaussian-KDE logsumexp kernel for Trainium2 (8 NeuronCores, SPMD).

Math: out[t] = ln Σ_n exp(-0.5·scale[n]·dist2[t,n] - Z), dist2 via the GEMM
expansion folded into ONE K=66 matmul:
    xhat[:, t] = [test_t (64), test_sq_t, 1]
    yhat[:, n] = [scale_n·train_n (64), -.5·scale_n, -.5·scale_n·train_sq_n - Z]
Weights ~ U[0,1] keep every exp-argument ≤ ~0, so no per-row max pass; the
per-point sum lands ~e^-61, comfortably inside fp32 normal range.

Sharding (t4n2): test split 4 ways (512/core, 4 P-tiles of 128), train split
2 ways (2048/core). Each core returns raw partial sums [128, 4]; the host
adds the two train-shard partials and takes log in float64 — no ln / reduce
on device, and the large 2e-2 harness tolerance lets the matmul run in bf16
(measured 1.2e-3 end-to-end) which halves input DMA bytes at the same
1 cycle/row PE streaming rate as float32r.

Per-core schedule:
  - Inputs split into x + 4 y-blocks, interleaved across the two HWDGE
    queues (sync / scalar) so issue overheads overlap and the first matmul
    starts as early as possible.
  - PE: per t-tile, 4 single K=66 matmuls of 512 cols into a [128,2048]
    PSUM buffer (4 banks), double-buffered across tiles.
  - ACT: one exp ACTIVATE per tile over the whole [128,2048] chunk with
    free-dim accumulation -> sums[:, t] is the finished partial sum; the
    result DMA is issued from the scalar queue right after the last
    accumulator read (no cross-engine handoff).
  - Vector waits for the result DMA's semaphore and clears all semaphores
    (iteration hygiene).
"""

import math
from contextlib import ExitStack

import numpy as np
import ml_dtypes

import concourse.bacc as bacc
import concourse.bass as bass
import concourse.mybir as mybir
from concourse.bass_utils import run_bass_kernel_spmd

N_CORES = 8
NT, NTR, D = 2048, 4096, 64
T_WAYS, N_WAYS = 4, 2
TPC = NT // T_WAYS           # 512 test points per core
NPC = NTR // N_WAYS          # 2048 train points per core
KA = D + 2                   # 66: augmented contraction dim
P = 128                      # partition tile of test points
T_TILES = TPC // P           # 4
MM_N = 512                   # matmul free-dim (one PSUM bank, fp32)
CHUNK = 2048                 # free dim of one exp ACTIVATE (= NPC)
NYB = NPC // MM_N            # 4 y DMA blocks
F32 = mybir.dt.float32
Z_CONST = float(0.5 * D * math.log(2.0 * math.pi) + math.log(NTR))  # h = 1

MM_DTYPE = mybir.dt.bfloat16
NP_DTYPE = {mybir.dt.bfloat16: ml_dtypes.bfloat16,
            mybir.dt.float32r: np.float32,
            mybir.dt.float32: np.float32}


def build_program_v2(mm_dtype=MM_DTYPE):
    nc = bacc.Bacc("TRN2")
    xh = nc.declare_dram_parameter("xh", [KA, TPC], mm_dtype, isOutput=False)
    yhs = [
        nc.declare_dram_parameter(f"yh{b}", [KA, MM_N], mm_dtype, isOutput=False)
        for b in range(NYB)
    ]
    out_d = nc.declare_dram_parameter("out", [P, T_TILES], F32, isOutput=True)

    with ExitStack() as ctx:
        sb = lambda nm, shape, dt: ctx.enter_context(nc.sbuf_tensor(nm, shape, dt))
        xs = sb("xs", [KA, TPC], mm_dtype)
        ys = [sb(f"ys{b}", [KA, MM_N], mm_dtype) for b in range(NYB)]
        et = [sb(f"et{k}", [P, CHUNK], F32) for k in range(2)]
        dummy_in = sb("dummy_in", [P, 1], F32)
        dummy_out = sb("dummy_out", [P, 1], F32)
        sums = sb("sums", [P, T_TILES], F32)
        pt = [
            ctx.enter_context(nc.psum_tensor(f"pt{k}", [P, CHUNK], F32))
            for k in range(2)
        ]

        sx = ctx.enter_context(nc.semaphore("sx"))
        sy = [ctx.enter_context(nc.semaphore(f"sy{b}")) for b in range(NYB)]
        spe = ctx.enter_context(nc.semaphore("spe"))
        sact = ctx.enter_context(nc.semaphore("sact"))
        so = ctx.enter_context(nc.semaphore("so"))
        my_sems = [sx, *sy, spe, sact, so]

        # Input DMAs, issued at engine boot, interleaved across both HWDGE
        # queues so the first blocks land earliest.
        nc.sync.dma_start(out=xs[:], in_=xh[:]).then_inc(sx, 16)
        nc.scalar.dma_start(out=ys[0][:], in_=yhs[0][:]).then_inc(sy[0], 16)
        nc.sync.dma_start(out=ys[1][:], in_=yhs[1][:]).then_inc(sy[1], 16)
        nc.scalar.dma_start(out=ys[2][:], in_=yhs[2][:]).then_inc(sy[2], 16)
        nc.sync.dma_start(out=ys[3][:], in_=yhs[3][:]).then_inc(sy[3], 16)

        # ACT: dummy exp triggers the activation-table load at boot
        nc.scalar.activation(
            dummy_out[:], dummy_in[:], mybir.ActivationFunctionType.Exp
        )

        # PE stream: per tile, 4 single K=66 matmuls
        for t in range(T_TILES):
            for j in range(NYB):
                if j == 0 and t >= 2:
                    nc.tensor.wait_ge(sact, t - 1)  # PSUM buf recycled
                if t == 0:
                    if j == 0:
                        nc.tensor.wait_ge(sx, 16)
                    nc.tensor.wait_ge(sy[j], 16)
                mm = nc.tensor.matmul(
                    pt[t % 2][:, j * MM_N:(j + 1) * MM_N],
                    xs[:, t * P:(t + 1) * P],
                    ys[j][:],
                    start=True,
                    stop=True,
                )
            mm.then_inc(spe, 1)

        # ACT stream: one exp + free-dim accumulation per tile; sums[:, t]
        # is the finished partial sum for that tile's 128 test points.
        for t in range(T_TILES):
            nc.scalar.wait_ge(spe, t + 1)
            nc.scalar.activation(
                out=et[t % 2][:],
                in_=pt[t % 2][:],
                func=mybir.ActivationFunctionType.Exp,
                accum_out=sums[:, t:t + 1],
            ).then_inc(sact, 1)

        # Result DMA straight from the scalar queue. The explicit wait is
        # required even on the issuing queue: the accumulator read-out is a
        # trailing micro-op of the ACTIVATE, and an un-gated DMA dispatch
        # races it; the semaphore only fires once the read-out has landed.
        nc.scalar.wait_ge(sact, T_TILES)
        nc.scalar.dma_start(out=out_d[:], in_=sums[:]).then_inc(so, 16)

        nc.vector.wait_ge(so, 16)
        sem_nums = sorted(s.num for s in my_sems)
        assert sem_nums == list(range(sem_nums[0], sem_nums[0] + len(sem_nums)))
        nc.vector.sem_clear(range(sem_nums[0], sem_nums[-1] + 1))

    nc.compile()
    _strip_preamble(nc)
    return nc


def _strip_preamble(nc):
    """Drop the framework's boot barrier (per-engine drain + event sems) and
    const-AP memsets — nothing reads the const APs and every engine can start
    immediately."""
    blk = nc.main_func.blocks[0]
    insts = list(blk.instructions)
    drop = set()
    for k, inst in enumerate(insts):
        tn = type(inst).__name__
        if tn == "InstEventSemaphore" and inst.name.startswith("barrier_"):
            drop.add(inst.name)
            if k > 0 and type(insts[k - 1]).__name__ == "InstDrain":
                drop.add(insts[k - 1].name)
        elif tn == "InstMemset" and inst.outs and "const-" in str(inst.outs[0]):
            drop.add(inst.name)
    blk.instructions[:] = [i for i in insts if i.name not in drop]


_PROG = {}


def _get_prog(mm_dtype=MM_DTYPE):
    if mm_dtype not in _PROG:
        _PROG[mm_dtype] = build_program_v2(mm_dtype)
    return _PROG[mm_dtype]


def _prepare(test_Xs, train_Xs, weights, np_dtype):
    test_Xs = np.asarray(test_Xs, dtype=np.float32)
    train_Xs = np.asarray(train_Xs, dtype=np.float32)
    weights = np.asarray(weights, dtype=np.float32)

    test_sq = (test_Xs.astype(np.float64) ** 2).sum(1)
    train_sq = (train_Xs.astype(np.float64) ** 2).sum(1)
    scale = weights.astype(np.float64) ** 2

    xhat = np.empty((KA, NT), np.float32)
    xhat[:D] = test_Xs.T
    xhat[D] = test_sq
    xhat[D + 1] = 1.0

    yhat = np.empty((KA, NTR), np.float32)
    yhat[:D] = (train_Xs.astype(np.float64) * scale[:, None]).T
    yhat[D] = -0.5 * scale
    yhat[D + 1] = -0.5 * scale * train_sq - Z_CONST
    return xhat.astype(np_dtype), yhat.astype(np_dtype)


def kernel(test_Xs, train_Xs, weights, mm_dtype=MM_DTYPE, trace=False):
    xhat, yhat = _prepare(test_Xs, train_Xs, weights, NP_DTYPE[mm_dtype])
    nc = _get_prog(mm_dtype)
    in_maps = []
    for c in range(N_CORES):
        tc, nc2 = divmod(c, N_WAYS)
        m = {"xh": np.ascontiguousarray(xhat[:, tc * TPC:(tc + 1) * TPC])}
        for b in range(NYB):
            lo = nc2 * NPC + b * MM_N
            m[f"yh{b}"] = np.ascontiguousarray(yhat[:, lo:lo + MM_N])
        in_maps.append(m)
    res = run_bass_kernel_spmd(nc, in_maps, list(range(N_CORES)), trace=trace)
    # res[c]["out"] is [128, T_TILES] of partial sums; combine the N_WAYS
    # train shards per test-slice on the host, then log (float64).
    out = np.empty(NT, np.float64)
    for tc in range(T_WAYS):
        tot = np.zeros((P, T_TILES), np.float64)
        for nc2 in range(N_WAYS):
            tot += res.results[tc * N_WAYS + nc2]["out"].astype(np.float64)
        out[tc * TPC:(tc + 1) * TPC] = np.log(tot).T.ravel()
    if trace:
        kernel.last_results = res
    return out.astype(np.float32)
